# revision 45
# baseline (speedup 1.0000x reference)
"""Trainium2 Bass kernel for APNet2 dAPNet2 MPNN readout + segment reduce.

Computes, for 500k atom pairs:
    E = MLP(hAB) + MLP(hBA)          (4-layer MLP, shared weights)
    delta_E = E * cutoff
    out = segment_sum(delta_E, dimer_ind, ndimer=2048)   -> [2048, 1]

Strategy (8 NeuronCores, data-parallel over pairs):
  - Host: pad pairs to 8*62976, shard across cores; pre-transpose hAB/hBA to
    [128 feat, P] bf16 so the MLP runs in "transposed activation" layout
    (features on partitions, pairs on the free dim) with zero on-chip
    transposes.
  - Device (per core): per 512-pair tile, bf16 matmuls through the MLP in
    transposed layout; the final linear layer is computed per 128-pair
    chunk with the summed relu(Y3) as the stationary operand, giving
    per-pair energies E on partitions. Segment reduce uses a two-level
    one-hot decomposition d = 32*q + r (q in [0,64), r in [0,32)) with the
    one-hot matrices built on the host (U = onehot(q), Vc = onehot(r) *
    cutoff, both DMA'd per tile):
       seg[r, q] += sum_p (Vc[p,r] * E[p]) * U[p,q]
    as one PE matmul per chunk into a PSUM tile, accumulated in SBUF.
  - Host: out[32q+r] = sum_cores seg_core[r, q] (+ the linear 2*b4 term).
"""

import numpy as np
import ml_dtypes

BF = ml_dtypes.bfloat16

NCORES = 8
N_PAIRS = 500_000
FEAT = 128
NDIMER = 2048
H1, H2, H3 = 256, 128, 64
TILE = 512          # pairs per tile (free dim)
CH = 4              # 128-pair chunks per tile

# per-core pair count (padded), must be divisible by TILE
PC = 62976          # = 512 * 123 ; 8 * 62976 = 503808 >= 500000
NT = PC // TILE     # 123


_NC_CACHE = {}


def _build_nc(nt: int):
    """Build the per-core Bass graph for nt tiles of 512 pairs."""
    from concourse import bacc, mybir, tile

    f32 = mybir.dt.float32
    bf16 = mybir.dt.bfloat16
    RELU = mybir.ActivationFunctionType.Relu
    ADD = mybir.AluOpType.add
    MAX = mybir.AluOpType.max
    MULT = mybir.AluOpType.mult

    nc = bacc.Bacc("TRN2", target_bir_lowering=False, debug=False)

    COPY = mybir.ActivationFunctionType.Copy

    pc = nt * TILE
    xab_e = nc.declare_dram_parameter("xab", [FEAT, pc], bf16, isOutput=False)
    xba_e = nc.declare_dram_parameter("xba", [FEAT, pc], bf16, isOutput=False)
    # per-tile one-hot matrices, host-built: U (pair x 64 q-bins per chunk),
    # Vc (pair x 32 r-bins per chunk, pre-multiplied by cutoff)
    u_e = nc.declare_dram_parameter("useg", [nt, 128, CH * 64], bf16,
                                    isOutput=False)
    vc_e = nc.declare_dram_parameter("vseg", [nt, 128, CH * 32], bf16,
                                     isOutput=False)
    w1_e = nc.declare_dram_parameter("w1", [FEAT, H1], bf16, isOutput=False)
    w2_e = nc.declare_dram_parameter("w2", [H1, H2], bf16, isOutput=False)
    w3_e = nc.declare_dram_parameter("w3", [H2, H3], bf16, isOutput=False)
    w4_e = nc.declare_dram_parameter("w4", [128, 1], bf16, isOutput=False)
    b1_e = nc.declare_dram_parameter("b1", [128, 2], f32, isOutput=False)
    b2_e = nc.declare_dram_parameter("b2", [128, 1], f32, isOutput=False)
    b3_e = nc.declare_dram_parameter("b3", [64, 1], f32, isOutput=False)
    out_e = nc.declare_dram_parameter("out", [32, 64], f32, isOutput=True)

    with tile.TileContext(nc) as tc:
        with (
            tc.tile_pool(name="const", bufs=1) as cpool,
            tc.tile_pool(name="xpool", bufs=6) as xpool,
            tc.tile_pool(name="mpool", bufs=6) as mpool,
            tc.tile_pool(name="act", bufs=3) as apool,
            tc.tile_pool(name="seg", bufs=6) as gpool,
            tc.tile_pool(name="ps_y1", bufs=1, space="PSUM") as ps_y1,
            tc.tile_pool(name="ps_y2", bufs=1, space="PSUM") as ps_y2,
            tc.tile_pool(name="ps_y3", bufs=1, space="PSUM") as ps_y3,
            tc.tile_pool(name="ps_e", bufs=1, space="PSUM") as ps_e,
        ):
            # ---- constants (loaded once) ----
            w1s = cpool.tile([FEAT, H1], bf16, tag="w1s")
            nc.sync.dma_start(out=w1s[:], in_=w1_e[:, :])
            w2s0 = cpool.tile([128, H2], bf16, tag="w2s0")
            nc.sync.dma_start(out=w2s0[:], in_=w2_e[0:128, :])
            w2s1 = cpool.tile([128, H2], bf16, tag="w2s1")
            nc.sync.dma_start(out=w2s1[:], in_=w2_e[128:256, :])
            w3s = cpool.tile([H2, H3], bf16, tag="w3s")
            nc.sync.dma_start(out=w3s[:], in_=w3_e[:, :])
            w4s = cpool.tile([128, 1], bf16, tag="w4s")
            nc.sync.dma_start(out=w4s[:], in_=w4_e[:, :])
            b1s = cpool.tile([128, 2], f32, tag="b1s")
            nc.sync.dma_start(out=b1s[:], in_=b1_e[:, :])
            b2s = cpool.tile([128, 1], f32, tag="b2s")
            nc.sync.dma_start(out=b2s[:], in_=b2_e[:, :])
            b3s = cpool.tile([64, 1], f32, tag="b3s")
            nc.sync.dma_start(out=b3s[:], in_=b3_e[:, :])

            # SBUF accumulator for the segment partials
            segacc = cpool.tile([32, 64], f32, tag="segacc")
            nc.vector.memset(segacc[:], 0.0)

            for t in range(nt):
                xa = xpool.tile([FEAT, TILE], bf16, tag="xa")
                nc.sync.dma_start(out=xa[:], in_=xab_e[:, t * TILE:(t + 1) * TILE])
                xb = xpool.tile([FEAT, TILE], bf16, tag="xb")
                nc.sync.dma_start(out=xb[:], in_=xba_e[:, t * TILE:(t + 1) * TILE])
                ut = mpool.tile([128, CH * 64], bf16, tag="ut")
                nc.sync.dma_start(out=ut[:], in_=u_e[t, :, :])
                vt = mpool.tile([128, CH * 32], bf16, tag="vt")
                nc.sync.dma_start(out=vt[:], in_=vc_e[t, :, :])

                # ---- L1: both h's share each stationary weight chunk;
                # [AB | BA] packed on the free dim of one 2-bank PSUM tile
                y1a = ps_y1.tile([128, 2 * TILE], f32, tag="y1a")
                nc.tensor.matmul(out=y1a[:, 0:TILE], lhsT=w1s[:, 0:128],
                                 rhs=xa[:], start=True, stop=True)
                nc.tensor.matmul(out=y1a[:, TILE:2 * TILE], lhsT=w1s[:, 0:128],
                                 rhs=xb[:], start=True, stop=True)
                y1b = ps_y1.tile([128, 2 * TILE], f32, tag="y1b")
                nc.tensor.matmul(out=y1b[:, 0:TILE], lhsT=w1s[:, 128:256],
                                 rhs=xa[:], start=True, stop=True)
                nc.tensor.matmul(out=y1b[:, TILE:2 * TILE], lhsT=w1s[:, 128:256],
                                 rhs=xb[:], start=True, stop=True)
                y1sa = apool.tile([128, 2 * TILE], bf16, tag="y1sa")
                nc.scalar.activation(out=y1sa[:], in_=y1a[:], func=RELU,
                                     bias=b1s[:, 0:1])
                y1sb = apool.tile([128, 2 * TILE], bf16, tag="y1sb")
                nc.scalar.activation(out=y1sb[:], in_=y1b[:], func=RELU,
                                     bias=b1s[:, 1:2])

                # ---- L2: both h halves into one 2-bank PSUM tile, one evac
                y2p = ps_y2.tile([128, 2 * TILE], f32, tag="y2p")
                for hi in (0, 1):
                    sl = slice(hi * TILE, (hi + 1) * TILE)
                    nc.tensor.matmul(out=y2p[:, sl], lhsT=w2s0[:],
                                     rhs=y1sa[:, sl], start=True, stop=False)
                    nc.tensor.matmul(out=y2p[:, sl], lhsT=w2s1[:],
                                     rhs=y1sb[:, sl], start=False, stop=True)
                y2s = apool.tile([128, 2 * TILE], bf16, tag="y2s")
                nc.vector.tensor_scalar(out=y2s[:], in0=y2p[:],
                                        scalar1=b2s[:, 0:1], scalar2=0.0,
                                        op0=ADD, op1=MAX)

                # ---- L3 (single-bank PSUM reused AB -> BA)
                t3list = []
                for hi in (0, 1):
                    sl = slice(hi * TILE, (hi + 1) * TILE)
                    y3p = ps_y3.tile([H3, TILE], f32, tag="y3p")
                    nc.tensor.matmul(out=y3p[:], lhsT=w3s[:], rhs=y2s[:, sl],
                                     start=True, stop=True)
                    t3 = apool.tile([H3, TILE], bf16, tag=f"t3{hi}")
                    if hi == 0:
                        nc.scalar.activation(out=t3[:], in_=y3p[:],
                                             func=RELU, bias=b3s[:, 0:1])
                    else:
                        nc.vector.tensor_scalar(out=t3[:], in0=y3p[:],
                                                scalar1=b3s[:, 0:1], scalar2=0.0,
                                                op0=ADD, op1=MAX)
                    t3list.append(t3)

                # S = relu(Y3_AB) + relu(Y3_BA)  (L4 is linear, so one matmul)
                ss = apool.tile([64, TILE], bf16, tag="ss")
                nc.gpsimd.tensor_tensor(out=ss[:], in0=t3list[0][:],
                                        in1=t3list[1][:], op=ADD)

                # ---- L4: E[p] = S[:,chunk]^T @ W4
                ep = ps_e.tile([128, CH], f32, tag="epseg")
                for c in range(CH):
                    nc.tensor.matmul(out=ep[:, c:c + 1],
                                     lhsT=ss[:, c * 128:(c + 1) * 128],
                                     rhs=w4s[0:64, :], start=True, stop=True,
                                     skip_group_check=True)
                et = apool.tile([128, CH], f32, tag="et")
                nc.scalar.activation(out=et[:], in_=ep[:], func=COPY, bias=0.0)

                # aa[p, (c,r)] = Vc[p, (c,r)] * E[p, c]  -- one fused op via a
                # broadcast AP on the chunk axis
                aa = gpool.tile([128, CH * 32], bf16, tag="aa")
                nc.vector.tensor_tensor(
                    out=aa[:].rearrange("p (c r) -> p c r", r=32),
                    in0=vt[:].rearrange("p (c r) -> p c r", r=32),
                    in1=et[:].broadcast_to([128, CH, 32]),
                    op=MULT)

                # ---- segment reduce: segp[r, q] += A_c^T @ U_c  (d = 32q + r)
                segp = ps_e.tile([32, 64], f32, tag="epseg")
                for c in range(CH):
                    nc.tensor.matmul(out=segp[:],
                                     lhsT=aa[:, c * 32:(c + 1) * 32],
                                     rhs=ut[:, c * 64:(c + 1) * 64],
                                     start=(c == 0), stop=(c == CH - 1),
                                     skip_group_check=True)
                nc.vector.tensor_tensor(out=segacc[:], in0=segacc[:],
                                        in1=segp[:], op=ADD)

            nc.sync.dma_start(out=out_e[:, :], in_=segacc[:])

    nc.finalize()
    return nc


def _get_nc(nt: int):
    if nt not in _NC_CACHE:
        _NC_CACHE[nt] = _build_nc(nt)
    return _NC_CACHE[nt]


def _prep_core_inputs(hAB16, hBA16, cutoff, qv, rv, weights, lo, hi, pc):
    """Build the in_map for one core covering global pairs [lo, hi)."""
    nt = pc // TILE
    n = hi - lo
    xa = np.zeros((FEAT, pc), dtype=BF)
    xa[:, :n] = hAB16[lo:hi].T
    xb = np.zeros((FEAT, pc), dtype=BF)
    xb[:, :n] = hBA16[lo:hi].T

    qq = np.zeros((pc,), dtype=np.int32)
    qq[:n] = qv[lo:hi]
    rr = np.full((pc,), -1, dtype=np.int32)  # padded pairs match no r-bin
    rr[:n] = rv[lo:hi]
    cc = np.zeros((pc,), dtype=np.float32)
    cc[:n] = cutoff[lo:hi]

    # u[t, p, c*64 + j] = (q of pair (t*512 + c*128 + p) == j)
    Q = qq.reshape(nt, CH, 128)
    useg = (Q[..., None] == np.arange(64, dtype=np.int32))
    useg = useg.transpose(0, 2, 1, 3).reshape(nt, 128, CH * 64).astype(BF)
    # vc[t, p, c*32 + j] = (r == j) * cutoff  (cutoff pre-folded)
    R = rr.reshape(nt, CH, 128)
    C = cc.reshape(nt, CH, 128)
    vseg = (R[..., None] == np.arange(32, dtype=np.int32)) * C[..., None]
    vseg = vseg.transpose(0, 2, 1, 3).reshape(nt, 128, CH * 32).astype(BF)

    m = {"xab": xa, "xba": xb, "useg": useg, "vseg": vseg}
    m.update(weights)
    return m


class _Runner:
    """Reusable SPMD executor for a built Bass graph (mirrors
    bass2jax.run_bass_via_pjrt's multi-core path, but keeps the jitted
    callable and device-resident inputs so executions can be repeated and
    timed)."""

    def __init__(self, nc, ncores):
        import jax
        from jax.sharding import Mesh, PartitionSpec, NamedSharding
        from jax.experimental.shard_map import shard_map
        from concourse import bass2jax, mybir

        bass2jax.install_neuronx_cc_hook()
        self.ncores = ncores
        partition_name = (nc.partition_id_tensor.name
                          if nc.partition_id_tensor else None)
        in_names, out_names, out_avals, zero_outs = [], [], [], []
        for alloc in nc.m.functions[0].allocations:
            if not isinstance(alloc, mybir.MemoryLocationSet):
                continue
            name = alloc.memorylocations[0].name
            if alloc.kind == "ExternalInput":
                if name != partition_name:
                    in_names.append(name)
            elif alloc.kind == "ExternalOutput":
                out_names.append(name)
                shape = tuple(alloc.tensor_shape)
                dtype = mybir.dt.np(alloc.dtype)
                out_avals.append(jax.core.ShapedArray(shape, dtype))
                zero_outs.append(np.zeros((ncores * shape[0], *shape[1:]), dtype))
        self.in_names = list(in_names)
        self.out_names = list(out_names)
        self.out_avals = out_avals
        self.zero_outs = zero_outs
        n_params = len(in_names)
        all_in_names = in_names + out_names
        if partition_name is not None:
            all_in_names = all_in_names + [partition_name]

        def _bind(operands):
            if partition_name is not None:
                operands = operands + [bass2jax.partition_id_tensor()]
            return bass2jax._bass_exec_p.bind(
                *operands,
                out_avals=tuple(out_avals),
                in_names=tuple(all_in_names),
                out_names=tuple(out_names),
                lowering_input_output_aliases=(),
                sim_require_finite=True,
                sim_require_nnan=True,
                nc=nc,
            )

        def _make_fn(k):
            def _body(*args):
                ins = list(args[:n_params])
                zouts = tuple(args[n_params:])
                if k == 1:
                    return tuple(_bind(ins + list(zouts)))

                # repeat the NEFF k times in one dispatch via lax.scan,
                # chaining outputs into the next rep's out-buffers (the
                # compile hook allows only one bass_exec per HLO module)
                def step(carry, _):
                    return tuple(_bind(ins + list(carry))), ()

                carry, _ = jax.lax.scan(step, zouts, None, length=k)
                return tuple(carry)

            return jax.jit(
                shard_map(_body, mesh=self.mesh, in_specs=in_specs,
                          out_specs=out_specs, check_rep=False),
                donate_argnums=tuple(range(n_params, n_params + len(out_names))),
                keep_unused=True,
            )

        devices = jax.devices()[:ncores]
        self.mesh = Mesh(np.asarray(devices), ("core",))
        self.sharding = NamedSharding(self.mesh, PartitionSpec("core"))
        in_specs = (PartitionSpec("core"),) * (n_params + len(out_names))
        out_specs = (PartitionSpec("core"),) * len(out_names)
        self._make_fn = _make_fn
        self._fns = {}
        self.fn = self._get_fn(1)
        self.dev_in = None

    def _get_fn(self, k):
        if k not in self._fns:
            self._fns[k] = self._make_fn(k)
        return self._fns[k]

    def load_inputs(self, in_maps):
        import jax
        concat = [
            np.concatenate([np.asarray(in_maps[c][k]) for c in range(self.ncores)],
                           axis=0)
            for k in self.in_names
        ]
        self.dev_in = [jax.device_put(a, self.sharding) for a in concat]

    def run(self):
        import jax
        zeros = [np.zeros_like(z) for z in self.zero_outs]
        outs = self.fn(*self.dev_in, *zeros)
        outs = [np.asarray(o) for o in outs]
        return [
            {name: outs[i].reshape(self.ncores, *self.out_avals[i].shape)[c]
             for i, name in enumerate(self.out_names)}
            for c in range(self.ncores)
        ]

    def bench(self, n=5, k=1):
        import time, jax
        fn = self._get_fn(k)
        times = []
        for _ in range(n):
            zeros = [np.zeros_like(z) for z in self.zero_outs]
            t0 = time.perf_counter()
            outs = fn(*self.dev_in, *zeros)
            jax.block_until_ready(outs)
            times.append(time.perf_counter() - t0)
        return times

    def bench_exec_ns(self, n=6, k1=1, k2=17):
        """Per-NEFF-execution time via differential timing: k2 vs k1 chained
        executions inside one dispatch cancels the (large) dispatch overhead."""
        t1 = self.bench(n=n, k=k1)
        t2 = self.bench(n=n, k=k2)
        per = (min(t2) - min(t1)) / (k2 - k1)
        return per * 1e9, t1, t2


_RUNNER_CACHE = {}
LAST_RUNNER = None
LAST_IN_MAPS = None
LAST_NT = None


def _get_runner(nt, ncores):
    key = (nt, ncores)
    if key not in _RUNNER_CACHE:
        _RUNNER_CACHE[key] = _Runner(_get_nc(nt), ncores)
    return _RUNNER_CACHE[key]


def _run(hAB, hBA, cutoff, dimer_ind, W1, b1, W2, b2, W3, b3, W4, b4,
         pc=PC, ncores=NCORES):
    global LAST_RUNNER

    nt = pc // TILE
    hAB16 = np.ascontiguousarray(np.asarray(hAB, dtype=np.float32)).astype(BF)
    hBA16 = np.ascontiguousarray(np.asarray(hBA, dtype=np.float32)).astype(BF)
    cutoff = np.asarray(cutoff, dtype=np.float32).reshape(-1)
    dimer = np.asarray(dimer_ind).astype(np.int64).reshape(-1)
    qv = (dimer >> 5).astype(np.int32)
    rv = (dimer & 31).astype(np.int32)

    b1 = np.asarray(b1, np.float32)
    b4 = np.asarray(b4, np.float32).reshape(-1)
    w4d = np.asarray(W4, np.float32).astype(BF).reshape(H3, 1)
    weights = {
        "w1": np.asarray(W1, np.float32).astype(BF),
        "w2": np.asarray(W2, np.float32).astype(BF),
        "w3": np.asarray(W3, np.float32).astype(BF),
        "w4": np.ascontiguousarray(np.tile(w4d, (2, 1))),
        "b1": np.ascontiguousarray(b1.reshape(2, 128).T),
        "b2": np.asarray(b2, np.float32).reshape(128, 1),
        "b3": np.asarray(b3, np.float32).reshape(64, 1),
    }

    npairs = hAB16.shape[0]
    in_maps = []
    for c in range(ncores):
        lo = min(c * pc, npairs)
        hi = min((c + 1) * pc, npairs)
        in_maps.append(
            _prep_core_inputs(hAB16, hBA16, cutoff, qv, rv, weights, lo, hi, pc)
        )

    runner = _get_runner(nt, ncores)
    runner.load_inputs(in_maps)
    LAST_RUNNER = runner
    global LAST_IN_MAPS, LAST_NT
    LAST_IN_MAPS = in_maps
    LAST_NT = nt
    results = runner.run()

    out = np.zeros((NDIMER, 1), dtype=np.float32)
    for c in range(ncores):
        part = np.asarray(results[c]["out"], dtype=np.float32)  # [32, 64]
        out[:, 0] += part.T.reshape(-1)  # d = q*32 + r
    if float(b4[0]) != 0.0:
        # b4 enters as (EAB+EBA+2*b4)*cutoff; the 2*b4*cutoff term is a pure
        # function of the inputs, accumulated on host
        bc = np.zeros((NDIMER,), np.float32)
        np.add.at(bc, dimer, cutoff)
        out[:, 0] += 2.0 * float(b4[0]) * bc
    return out


def kernel(**inputs):
    return _run(
        inputs["hAB"], inputs["hBA"], inputs["cutoff"], inputs["dimer_ind"],
        inputs["W1"], inputs["b1"], inputs["W2"], inputs["b2"],
        inputs["W3"], inputs["b3"], inputs["W4"], inputs["b4"],
    )
